# revision 13
# baseline (speedup 1.0000x reference)
"""Trainium2 Bass kernel for single-head full-dim attention (nn_CasualSelfAttention).

Reference math (B=4, S=4096, D=768, fp32):
    q = x @ Wq.T + bq ; k = x @ Wk.T + bk ; v = x @ Wv.T + bv
    att = softmax(q @ k.T * D**-0.5)        # no mask
    y = att @ v
    y = y.transpose(0,2,1).reshape(B,S,D)   # element permutation
    out = y @ Wc.T + bc

Sharding (8 cores): core c = 2*b + h handles batch b with ALL 4096 queries but
only its half of the keys/values (rows h*2048:(h+1)*2048). Each core produces a
partial unnormalized yT [768, 4096] (features x queries) plus partial softmax
sums. A pairwise ReduceScatter(add), chunked by 512-query column blocks, hands
core h the fully-reduced feature slice [384*h : 384*h+384] for all queries —
exactly the rows of y.T that the permutation maps to output rows
[2048*h : 2048*h+2048]. After normalizing by the (also-reduced) sums and adding
bv (valid because sum(att)=den), the flat buffer IS y_perm row-major, and the
final projection runs locally.

v2 performance structure:
  - Host pre-transposes x (and weights) so all device loads are plain DMAs.
  - q/k projections and the q.k^T matmul run in fp8e4m3 with DoubleRow
    (2 k-tiles of 128 contracted per matmul, ~1.4x tensor throughput). The
    fp8 weights/biases are pre-scaled by 32 on the host to stay in e4m3's
    normal range; the exp() activation scale absorbs the 32*32 factor.
  - av matmul, v/c projections stay bf16 (fp8 there would put ~2% error on
    the output, over the tolerance).
  - ReduceScatter payloads are bf16 (half the wire time); a tiny dummy
    collective at kernel start absorbs the first-collective warmup cost.
  - Softmax sums ride in the yTaug tensor (rows 384/769) so one RS reduces
    both; bv is folded in post-normalization (num/den + bv).
  - Norm for block b is emitted a few instructions into qc b+2 so its DVE ops
    can never stall phase C's exp->sums->av chain.
  - Phase F loads y_perm^T via 4 large transposed DMAs overlapped with its
    matmuls.
"""

import numpy as np
import ml_dtypes

BF16 = ml_dtypes.bfloat16
F8 = ml_dtypes.float8_e4m3

B, S, D = 4, 4096, 768
SK = S // 2            # keys per core
P = 128
DT = D // P            # 6 feature tiles
KT = SK // P           # 16 key tiles
QC = 512               # query chunk width
NQC = S // QC          # 8 query chunks / RS blocks
FH = D // 2            # 384: feature rows per RS chunk
WS = 32.0              # host-side fp8 weight scale for Wq/Wk (and bq/bk)
SCALE = float(D) ** -0.5
EXP_SCALE = SCALE / (WS * WS)
GROUPS = [[0, 1], [2, 3], [4, 5], [6, 7]]

_nc = None


def _build_program():
    import concourse.bass as bass
    import concourse.mybir as mybir
    import concourse.tile as tile
    from concourse import bacc

    f32 = mybir.dt.float32
    bf16 = mybir.dt.bfloat16
    f8 = mybir.dt.float8e4
    Exp = mybir.ActivationFunctionType.Exp
    Copy = mybir.ActivationFunctionType.Copy
    DR = mybir.MatmulPerfMode.DoubleRow

    nc = bacc.Bacc(None, num_devices=8)

    xq8 = nc.declare_dram_parameter("xq8", [D, S], f8, isOutput=False)
    xkv8 = nc.declare_dram_parameter("xkv8", [D, SK], f8, isOutput=False)
    xkv16 = nc.declare_dram_parameter("xkv16", [D, SK], bf16, isOutput=False)
    wq8 = nc.declare_dram_parameter("wq8", [D, D], f8, isOutput=False)
    wk8 = nc.declare_dram_parameter("wk8", [D, D], f8, isOutput=False)
    wvT = nc.declare_dram_parameter("wvT", [D, D], bf16, isOutput=False)
    wcT = nc.declare_dram_parameter("wcT", [D, D], bf16, isOutput=False)
    bq = nc.declare_dram_parameter("bq", [D, 1], f32, isOutput=False)
    bk = nc.declare_dram_parameter("bk", [D, 1], f32, isOutput=False)
    bvh = nc.declare_dram_parameter("bvh", [FH, 1], f32, isOutput=False)
    bc = nc.declare_dram_parameter("bc", [1, D], f32, isOutput=False)
    out = nc.declare_dram_parameter("out", [SK, D], f32, isOutput=True)

    def wload(dst, src):
        # [768, 768] row-major -> [128, 6, 768] with logical row g*128+p
        nc.sync.dma_start(dst[:], src[:].rearrange("(g p) d -> p g d", p=P))

    def xload(dst, src, c0, w):
        # [768, S] cols c0:c0+w -> [128, 6, w]
        nc.sync.dma_start(dst[:], src[:, c0:c0 + w].rearrange("(g p) s -> p g s", p=P))

    with tile.TileContext(nc) as tc:
        with tc.tile_pool(name="persist", bufs=1) as pp, \
             tc.tile_pool(name="dram", bufs=1, space="DRAM") as dram:
            # Per column block: rows 0:384 = feats 0:384, row 384 = partial
            # sums, rows 385:769 = feats 384:768, row 769 = partial sums.
            yTaug = [dram.tile([2 * (FH + 1), QC], bf16, name=f"yTaug{b}", tag=f"yTaug{b}")
                     for b in range(NQC)]
            rs_out = [dram.tile([FH + 1, QC], bf16, name=f"rs_out{b}", tag=f"rs_out{b}")
                      for b in range(NQC)]
            f_dram = dram.tile([SK, D], bf16)
            # realistically-sized dummy collective to absorb first-RS warmup
            # (contents uninitialized; output unused)
            warm_in = dram.tile([2 * (FH + 1), QC], bf16, name="warm_in", tag="warm_in")
            warm_out = dram.tile([FH + 1, QC], bf16, name="warm_out", tag="warm_out")

            # persistent SBUF. qT keeps each 512-query chunk's six feature
            # tiles contiguous: the att matmul's moving operand wants the
            # DoubleRow k-pair at a small (<=512B) stride.
            kT_sb = pp.tile([P, DT, SK], f8, tag="kT")
            qT_sb = pp.tile([P, NQC, DT, QC], f8, tag="qT")
            v_sb = [pp.tile([P, D], bf16, name=f"v{t}", tag=f"v{t}") for t in range(KT)]
            bq_sb = pp.tile([P, DT], f32, tag="bq_sb")
            bk_sb = pp.tile([P, DT], f32, tag="bk_sb")
            bvh_sb = pp.tile([P, 3], f32, tag="bvh_sb")
            ones_sb = pp.tile([P, P], bf16, name="ones", tag="ones")
            wc_sb = pp.tile([P, DT, D], bf16, tag="wc_sb")
            bc_sb = pp.tile([1, D], f32, tag="bc_sb")
            bcb = pp.tile([P, D], f32, tag="bcb")

            # ---- Phase A: kT [768x2048 fp8] and v [2048x768 bf16] ----
            import contextlib
            _ab_stack = contextlib.ExitStack()
            pa = _ab_stack.enter_context(tc.tile_pool(name="pA", bufs=1))
            with tc.tile_pool(name="psA", bufs=2, space="PSUM") as psa:
                wk_sb = pa.tile([P, DT, D], f8, tag="wk_sb")
                wload(wk_sb, wk8)
                x8cs = []
                for c in range(SK // QC):
                    x8c = pa.tile([P, DT, QC], f8, tag=f"x8c{c}", name=f"x8c{c}")
                    xload(x8c, xkv8, c * QC, QC)
                    x8cs.append(x8c)
                    if c == 0:
                        nc.sync.dma_start(
                            bk_sb[:], bk[:].rearrange("(g p) o -> p (g o)", p=P))
                wv_sb = pa.tile([P, DT, D], bf16, tag="wv_sb")
                wload(wv_sb, wvT)
                x16c0 = pa.tile([P, DT, QC], bf16, tag="x16c0", name="x16c0")
                xload(x16c0, xkv16, 0, QC)
                # all k-proj chunks back-to-back (pure fp8-DR, no mode mixing)
                for c in range(SK // QC):
                    for go in range(DT):
                        ps = psa.tile([P, QC], f32, tag="pk")
                        for a in range(3):
                            nc.tensor.matmul(
                                ps[:], wk_sb[:, 2 * a:2 * a + 2, go * P:(go + 1) * P],
                                x8cs[c][:, 2 * a:2 * a + 2, :],
                                start=(a == 0), stop=(a == 2), perf_mode=DR)
                        nc.vector.tensor_scalar_add(
                            kT_sb[:, go, c * QC:(c + 1) * QC], ps[:],
                            bk_sb[:, go:go + 1])
                # warm up the collectives stack while A computes
                nc.gpsimd.collective_compute(
                    "ReduceScatter", mybir.AluOpType.add,
                    replica_groups=GROUPS,
                    ins=[warm_in[:].opt()], outs=[warm_out[:].opt()])
                nc.vector.memset(ones_sb[:], 1.0)
                nc.sync.dma_start(bvh_sb[:], bvh[:].rearrange("(r p) o -> p (r o)", p=P))
                for c in range(SK // QC):
                    if c == 0:
                        x16c = x16c0
                    else:
                        x16c = pa.tile([P, DT, QC], bf16, tag="x16c", bufs=2, name="x16c")
                        xload(x16c, xkv16, c * QC, QC)
                    for tl in range(4):
                        t = c * 4 + tl
                        for half in range(2):
                            ps = psa.tile([P, FH], f32, tag="pv")
                            for gi in range(DT):
                                nc.tensor.matmul(
                                    ps[:], x16c[:, gi, tl * P:(tl + 1) * P],
                                    wv_sb[:, gi, half * FH:(half + 1) * FH],
                                    start=(gi == 0), stop=(gi == DT - 1))
                            nc.vector.tensor_copy(v_sb[t][:, half * FH:(half + 1) * FH], ps[:])

            # ---- Phase B: qT [768x4096 fp8] ----
            with tc.tile_pool(name="psB", bufs=3, space="PSUM") as psb:
                pb = pa
                wq_sb = pb.tile([P, DT, D], f8, tag="wq_sb")
                wload(wq_sb, wq8)
                nc.sync.dma_start(bq_sb[:], bq[:].rearrange("(g p) o -> p (g o)", p=P))
                wload(wc_sb, wcT)
                nc.sync.dma_start(bc_sb[:], bc[:])
                nc.gpsimd.partition_broadcast(bcb[:], bc_sb[:])
                for c in range(NQC):
                    x8c = pb.tile([P, DT, QC], f8, tag="xq8c", bufs=3, name="xq8c")
                    xload(x8c, xq8, c * QC, QC)
                    for go in range(DT):
                        ps = psb.tile([P, QC], f32, tag="pq")
                        for a in range(3):
                            nc.tensor.matmul(
                                ps[:], wq_sb[:, 2 * a:2 * a + 2, go * P:(go + 1) * P],
                                x8c[:, 2 * a:2 * a + 2, :],
                                start=(a == 0), stop=(a == 2), perf_mode=DR)
                        nc.vector.tensor_scalar_add(
                            qT_sb[:, c, go, :], ps[:],
                            bq_sb[:, go:go + 1])
                _ab_stack.close()

            # ---- Phase C: attention; yTaug; chunked RS; late norm ----
            with tc.tile_pool(name="pC", bufs=2) as pc, \
                 tc.tile_pool(name="pE", bufs=2) as pe:
                f_view = f_dram[:].rearrange("a b -> (a b)").rearrange(
                    "(x c) -> x c", c=S)

                # norm DMAs ride the scalar HWDGE queue: the sync queue stays
                # free for yTaug drains (C) and the fT transposes (F)
                def norm_head(b):
                    s_row = pe.tile([1, QC], bf16, tag="s_row", name="s_row")
                    nc.scalar.dma_start(s_row[:], rs_out[b][FH:FH + 1, :])
                    rsb = pe.tile([P, 3, QC], bf16, tag="rsb", bufs=2, name="rsb")
                    nc.scalar.dma_start(
                        rsb[:], rs_out[b][0:FH, :].rearrange("(r p) c -> p r c", p=P))
                    den = pe.tile([P, QC], bf16, tag="den", name="den")
                    nc.gpsimd.partition_broadcast(den[:], s_row[:])
                    den32 = pe.tile([P, QC], f32, tag="den32", name="den32")
                    nc.vector.tensor_copy(den32[:], den[:])
                    rec = pe.tile([P, QC], f32, tag="rec", name="rec", bufs=2)
                    nc.vector.reciprocal_approx_fast(rec[:], den32[:])
                    return rsb, rec

                def norm_r(b, rsb, rec, r):
                    tmp = pe.tile([P, QC], f32, tag="tmp", bufs=2, name="tmp")
                    nc.vector.tensor_mul(tmp[:], rsb[:, r, :], rec[:])
                    fn = pe.tile([P, QC], bf16, tag="fn", bufs=2, name="fn")
                    nc.vector.tensor_scalar_add(fn[:], tmp[:], bvh_sb[:, r:r + 1])
                    nc.scalar.dma_start(
                        f_view[r * P:(r + 1) * P, b * QC:(b + 1) * QC], fn[:])

                def emit_norm(b):
                    rsb, rec = norm_head(b)
                    for r in range(FH // P):
                        norm_r(b, rsb, rec, r)

                pending = []
                psc_ctx = tc.tile_pool(name="psC", bufs=1, space="PSUM")
                psc = psc_ctx.__enter__()
                for qc in range(NQC):
                    sums_acc = pc.tile([P, QC], bf16, tag="sums_acc")
                    nc.vector.memset(sums_acc[:], 0.0)
                    ypsum = [psc.tile([P, QC], f32, name=f"y{e}", tag=f"y{e}", bufs=1)
                             for e in range(DT)]
                    a_tiles = {}
                    for kt in range(KT):
                        if kt == 3 and pending:
                            emit_norm(pending.pop(0))
                        aps = psc.tile([P, QC], f32, tag="att", bufs=2)
                        for a in range(3):
                            nc.tensor.matmul(
                                aps[:], kT_sb[:, 2 * a:2 * a + 2, kt * P:(kt + 1) * P],
                                qT_sb[:, qc, 2 * a:2 * a + 2, :],
                                start=(a == 0), stop=(a == 2), perf_mode=DR)
                        # software pipeline: y-matmuls for kt-1 issue while
                        # the exp for kt is still on the scalar engine
                        if kt > 0:
                            for e in range(DT):
                                nc.tensor.matmul(
                                    ypsum[e][:], v_sb[kt - 1][:, e * P:(e + 1) * P],
                                    a_tiles[kt - 1][:],
                                    start=(kt - 1 == 0), stop=False)
                        a_sb = pc.tile([P, QC], bf16, tag="a_sb", bufs=6)
                        a_tiles[kt] = a_sb
                        nc.scalar.activation(a_sb[:], aps[:], Exp, scale=EXP_SCALE)
                        nc.vector.tensor_add(sums_acc[:], sums_acc[:], a_sb[:])
                    for e in range(DT):
                        nc.tensor.matmul(
                            ypsum[e][:], v_sb[KT - 1][:, e * P:(e + 1) * P],
                            a_tiles[KT - 1][:],
                            start=False, stop=True)
                    # ones.T @ sums_acc reduces across partitions and
                    # replicates the result onto all 128 partitions
                    sp = psc.tile([P, QC], f32, tag="att", bufs=2)
                    nc.tensor.matmul(sp[:], ones_sb[:], sums_acc[:], start=True, stop=True)
                    yb = yTaug[qc]
                    # drain ypsum banks in consumption order, alternating
                    # Scalar/Vector so the next qc's av matmuls free up fast
                    for e in range(DT):
                        yt_sb = pc.tile([P, QC], bf16, tag="yt_sb", bufs=4)
                        if e % 2 == 0:
                            nc.scalar.activation(yt_sb[:], ypsum[e][:], Copy)
                        else:
                            nc.vector.tensor_copy(yt_sb[:], ypsum[e][:])
                        row = e * P if e < 3 else (FH + 1) + (e - 3) * P
                        nc.sync.dma_start(yb[row:row + P, :], yt_sb[:])
                    sbc = pc.tile([P, QC], bf16, tag="sbc")
                    nc.vector.tensor_copy(sbc[:], sp[:])
                    nc.sync.dma_start(yb[FH:FH + 1, :], sbc[0:1, :])
                    nc.sync.dma_start(yb[2 * FH + 1:2 * FH + 2, :], sbc[0:1, :])

                    nc.gpsimd.collective_compute(
                        "ReduceScatter", mybir.AluOpType.add,
                        replica_groups=GROUPS,
                        ins=[yTaug[qc].opt()], outs=[rs_out[qc].opt()])
                    if qc > 0:
                        pending.append(qc - 1)

                psc_ctx.__exit__(None, None, None)
                for b in pending:
                    emit_norm(b)

                # ---- Phase F, pipelined with the last block's norm ----
                # fT block tb needs yT feature rows 96*tb..96*(tb+1): tb0 needs
                # only norm chunk r0, tb1 needs r0-r1, tb2/tb3 need r1-r2.
                with tc.tile_pool(name="pF", bufs=1) as pf, \
                     tc.tile_pool(name="psF", bufs=2, space="PSUM") as psf:
                    last = NQC - 1
                    rsb7, rec7 = norm_head(last)
                    norm_r(last, rsb7, rec7, 0)
                    for tb in range(SK // QC):
                        if tb in (1, 2):
                            norm_r(last, rsb7, rec7, tb)
                        fT = pf.tile([P, DT, QC], bf16, tag="fT", bufs=3, name="fT")
                        nc.sync.dma_start_transpose(
                            fT[:], f_dram[tb * QC:(tb + 1) * QC, :])
                        for u in range(4):
                            t = tb * 4 + u
                            po = psf.tile([P, QC], f32, tag="po")
                            po2 = psf.tile([P, D - QC], f32, tag="po2")
                            for gi in range(DT):
                                nc.tensor.matmul(po[:], fT[:, gi, u * P:(u + 1) * P],
                                                 wc_sb[:, gi, 0:QC],
                                                 start=(gi == 0), stop=(gi == DT - 1))
                                nc.tensor.matmul(po2[:], fT[:, gi, u * P:(u + 1) * P],
                                                 wc_sb[:, gi, QC:D],
                                                 start=(gi == 0), stop=(gi == DT - 1))
                            o_sb = pf.tile([P, D], f32, tag="o_sb", bufs=3)
                            nc.vector.tensor_add(o_sb[:, 0:QC], po[:], bcb[:, 0:QC])
                            nc.vector.tensor_add(o_sb[:, QC:D], po2[:], bcb[:, QC:D])
                            # out-writes go on the scalar HWDGE queue so the
                            # sync queue stays free for the fT transposes
                            nc.scalar.dma_start(out[t * P:(t + 1) * P, :], o_sb[:])

    return nc


def _get_nc():
    global _nc
    if _nc is None:
        _nc = _build_program()
        _nc.finalize()
    return _nc


def _to_f8(a):
    return np.clip(a, -240.0, 240.0).astype(F8)


def _prep_in_maps(x, Wq, bq, Wk, bk, Wv, bv, Wc, bc):
    x = np.asarray(x, dtype=np.float32)
    wq8 = _to_f8(np.ascontiguousarray(np.asarray(Wq, np.float32).T) * WS)
    wk8 = _to_f8(np.ascontiguousarray(np.asarray(Wk, np.float32).T) * WS)
    wvT = np.ascontiguousarray(np.asarray(Wv, np.float32).T).astype(BF16)
    wcT = np.ascontiguousarray(np.asarray(Wc, np.float32).T).astype(BF16)
    bqc = (np.asarray(bq, np.float32) * WS).reshape(D, 1).copy()
    bkc = (np.asarray(bk, np.float32) * WS).reshape(D, 1).copy()
    bvc = np.asarray(bv, np.float32).reshape(D)
    bcc = np.asarray(bc, np.float32).reshape(1, D).copy()
    in_maps = []
    for c in range(8):
        b, h = divmod(c, 2)
        xT = np.ascontiguousarray(x[b].T)          # [D, S]
        xT8 = _to_f8(xT)
        kv8 = np.ascontiguousarray(xT8[:, h * SK:(h + 1) * SK])
        kv16 = np.ascontiguousarray(xT[:, h * SK:(h + 1) * SK]).astype(BF16)
        in_maps.append({
            "xq8": xT8, "xkv8": kv8, "xkv16": kv16,
            "wq8": wq8, "wk8": wk8, "wvT": wvT, "wcT": wcT,
            "bq": bqc, "bk": bkc,
            "bvh": np.ascontiguousarray(bvc[h * FH:(h + 1) * FH]).reshape(FH, 1),
            "bc": bcc,
        })
    return in_maps


def _assemble(results):
    out = np.empty((B, S, D), dtype=np.float32)
    for c in range(8):
        b, h = divmod(c, 2)
        out[b, h * SK:(h + 1) * SK, :] = results[c]["out"]
    return out


def run_on_hw(trace=False, **inputs):
    from concourse.bass_utils import run_bass_kernel_spmd
    nc = _get_nc()
    in_maps = _prep_in_maps(**inputs)
    res = run_bass_kernel_spmd(nc, in_maps, list(range(8)), trace=trace)
    return _assemble(res.results), res


def kernel(**inputs):
    out, _ = run_on_hw(trace=False, **inputs)
    return out


# revision 14
# speedup vs baseline: 1.1474x; 1.1474x over previous
"""Trainium2 Bass kernel for single-head full-dim attention (nn_CasualSelfAttention).

Reference math (B=4, S=4096, D=768, fp32):
    q = x @ Wq.T + bq ; k = x @ Wk.T + bk ; v = x @ Wv.T + bv
    att = softmax(q @ k.T * D**-0.5)        # no mask
    y = att @ v
    y = y.transpose(0,2,1).reshape(B,S,D)   # element permutation
    out = y @ Wc.T + bc

Sharding (8 cores): core c = 2*b + h handles batch b with ALL 4096 queries but
only its half of the keys/values (rows h*2048:(h+1)*2048). Each core produces a
partial unnormalized yT [768, 4096] (features x queries) plus partial softmax
sums. A pairwise ReduceScatter(add), chunked by 512-query column blocks, hands
core h the fully-reduced feature slice [384*h : 384*h+384] for all queries —
exactly the rows of y.T that the permutation maps to output rows
[2048*h : 2048*h+2048]. After normalizing by the (also-reduced) sums and adding
bv (valid because sum(att)=den), the flat buffer IS y_perm row-major, and the
final projection runs locally.

Performance structure (v5):
  - All inputs host-pre-shuffled to [128-partition, ...] contiguous layouts so
    every device load is a max-rate plain DMA.
  - q/k projections, q.k^T, AND att@v run as fp8e4m3 DoubleRow matmuls
    (2 contraction k-tiles of 128 per matmul). The DR moving operand keeps its
    k-pair at a small stride (<=768B) — large j-strides halve the issue rate.
  - v/c projections stay bf16 (fp8 there pushes output error past the gate);
    v is computed in bf16 then quantized to fp8 for the av matmul.
  - ReduceScatter payloads are bf16; softmax sums ride in yTaug rows 384/769;
    a full-size dummy collective at kernel start absorbs first-RS warmup.
  - bv is folded in post-normalization (num/den + bv).
  - Norm for block b is emitted a few instructions into qc b+2, with its DMAs
    on the sync queue (scalar-queue loads would block the exp stream).
  - Tail: the last block's norm runs on the scalar queue right after the last
    RS, then phase F streams 4 transposed loads (bufs=4) against its matmuls.
"""

import numpy as np
import ml_dtypes

BF16 = ml_dtypes.bfloat16
F8 = ml_dtypes.float8_e4m3

B, S, D = 4, 4096, 768
SK = S // 2            # keys per core
P = 128
DT = D // P            # 6 feature tiles
KT = SK // P           # 16 key tiles
NPR = KT // 2          # 8 key-tile pairs for the av DoubleRow matmuls
QC = 512               # query chunk width
NQC = S // QC          # 8 query chunks / RS blocks
NKC = SK // QC         # 4 key chunks
FH = D // 2            # 384: feature rows per RS chunk
WS = 32.0              # host-side fp8 weight scale for Wq/Wk (and bq/bk)
SCALE = float(D) ** -0.5
EXP_SCALE = SCALE / (WS * WS)
GROUPS = [[0, 1], [2, 3], [4, 5], [6, 7]]

_nc = None


def _build_program():
    import concourse.bass as bass
    import concourse.mybir as mybir
    import concourse.tile as tile
    from concourse import bacc

    f32 = mybir.dt.float32
    bf16 = mybir.dt.bfloat16
    f8 = mybir.dt.float8e4
    Exp = mybir.ActivationFunctionType.Exp
    Copy = mybir.ActivationFunctionType.Copy
    DR = mybir.MatmulPerfMode.DoubleRow

    nc = bacc.Bacc(None, num_devices=8)

    # all x/weight layouts are host-pre-shuffled to partition-major flat 2D
    xq8 = nc.declare_dram_parameter("xq8", [P, NQC * DT * QC], f8, isOutput=False)
    xkv8 = nc.declare_dram_parameter("xkv8", [P, NKC * DT * QC], f8, isOutput=False)
    xkv16 = nc.declare_dram_parameter("xkv16", [P, NKC * DT * QC], bf16, isOutput=False)
    wq8 = nc.declare_dram_parameter("wq8", [P, DT * D], f8, isOutput=False)
    wk8 = nc.declare_dram_parameter("wk8", [P, DT * D], f8, isOutput=False)
    wvT = nc.declare_dram_parameter("wvT", [P, DT * D], bf16, isOutput=False)
    wcT = nc.declare_dram_parameter("wcT", [P, DT * D], bf16, isOutput=False)
    bq = nc.declare_dram_parameter("bq", [P, DT], f32, isOutput=False)
    bk = nc.declare_dram_parameter("bk", [P, DT], f32, isOutput=False)
    bvh = nc.declare_dram_parameter("bvh", [P, 3], f32, isOutput=False)
    bc = nc.declare_dram_parameter("bc", [1, D], f32, isOutput=False)
    out = nc.declare_dram_parameter("out", [SK, D], f32, isOutput=True)

    CHB = DT * QC  # 3072: one 512-col chunk of x per partition row

    with tile.TileContext(nc) as tc:
        with tc.tile_pool(name="persist", bufs=1) as pp, \
             tc.tile_pool(name="dram", bufs=1, space="DRAM") as dram:
            # Per column block: rows 0:384 = feats 0:384, row 384 = partial
            # sums, rows 385:769 = feats 384:768, row 769 = partial sums.
            yTaug = [dram.tile([2 * (FH + 1), QC], bf16, name=f"yTaug{b}", tag=f"yTaug{b}")
                     for b in range(NQC)]
            rs_out = [dram.tile([FH + 1, QC], bf16, name=f"rs_out{b}", tag=f"rs_out{b}")
                      for b in range(NQC)]
            f_dram = dram.tile([SK, D], bf16)
            # realistically-sized dummy collective to absorb first-RS warmup
            # (contents uninitialized; output unused)
            warm_in = dram.tile([2 * (FH + 1), QC], bf16, name="warm_in", tag="warm_in")
            warm_out = dram.tile([FH + 1, QC], bf16, name="warm_out", tag="warm_out")

            # persistent SBUF. qT keeps each 512-query chunk's six feature
            # tiles contiguous; v keeps the key-tile pairs at stride 768B.
            kT_sb = pp.tile([P, DT, SK], f8, tag="kT")
            qT_sb = pp.tile([P, NQC, DT, QC], f8, tag="qT")
            v_sb = pp.tile([P, KT, D], f8, tag="v_sb")
            bq_sb = pp.tile([P, DT], f32, tag="bq_sb")
            bk_sb = pp.tile([P, DT], f32, tag="bk_sb")
            bvh_sb = pp.tile([P, 3], f32, tag="bvh_sb")
            ones_sb = pp.tile([P, P], bf16, name="ones", tag="ones")
            wc_sb = pp.tile([P, DT, D], bf16, tag="wc_sb")
            bc_sb = pp.tile([1, D], f32, tag="bc_sb")
            bcb = pp.tile([P, D], f32, tag="bcb")

            # ---- Phase A: kT [768x2048 fp8] and v [2048x768 fp8] ----
            import contextlib
            _ab_stack = contextlib.ExitStack()
            pa = _ab_stack.enter_context(tc.tile_pool(name="pA", bufs=1))
            with tc.tile_pool(name="psA", bufs=2, space="PSUM") as psa:
                wk_sb = pa.tile([P, DT, D], f8, tag="wk_sb")
                nc.sync.dma_start(wk_sb[:], wk8[:])
                x8cs = []
                for c in range(NKC):
                    x8c = pa.tile([P, DT, QC], f8, tag=f"x8c{c}", name=f"x8c{c}")
                    nc.sync.dma_start(x8c[:], xkv8[:, c * CHB:(c + 1) * CHB])
                    x8cs.append(x8c)
                    if c == 0:
                        nc.sync.dma_start(bk_sb[:], bk[:])
                wv_sb = pa.tile([P, DT, D], bf16, tag="wv_sb")
                nc.sync.dma_start(wv_sb[:], wvT[:])
                x16c0 = pa.tile([P, DT, QC], bf16, tag="x16c0", name="x16c0")
                nc.sync.dma_start(x16c0[:], xkv16[:, 0:CHB])
                # all k-proj chunks back-to-back (pure fp8-DR, no mode mixing)
                for c in range(NKC):
                    for go in range(DT):
                        ps = psa.tile([P, QC], f32, tag="pk")
                        for a in range(3):
                            nc.tensor.matmul(
                                ps[:], wk_sb[:, 2 * a:2 * a + 2, go * P:(go + 1) * P],
                                x8cs[c][:, 2 * a:2 * a + 2, :],
                                start=(a == 0), stop=(a == 2), perf_mode=DR)
                        nc.vector.tensor_scalar_add(
                            kT_sb[:, go, c * QC:(c + 1) * QC], ps[:],
                            bk_sb[:, go:go + 1])
                # warm up the collectives stack while A computes
                nc.gpsimd.collective_compute(
                    "ReduceScatter", mybir.AluOpType.add,
                    replica_groups=GROUPS,
                    ins=[warm_in[:].opt()], outs=[warm_out[:].opt()])
                nc.vector.memset(ones_sb[:], 1.0)
                nc.sync.dma_start(bvh_sb[:], bvh[:])
                for c in range(NKC):
                    if c == 0:
                        x16c = x16c0
                    else:
                        x16c = pa.tile([P, DT, QC], bf16, tag="x16c", bufs=2, name="x16c")
                        nc.sync.dma_start(x16c[:], xkv16[:, c * CHB:(c + 1) * CHB])
                    for tl in range(4):
                        t = c * 4 + tl
                        for half in range(2):
                            ps = psa.tile([P, FH], f32, tag="pv")
                            for gi in range(DT):
                                nc.tensor.matmul(
                                    ps[:], x16c[:, gi, tl * P:(tl + 1) * P],
                                    wv_sb[:, gi, half * FH:(half + 1) * FH],
                                    start=(gi == 0), stop=(gi == DT - 1))
                            nc.vector.tensor_copy(
                                v_sb[:, t, half * FH:(half + 1) * FH], ps[:])

            # ---- Phase B: qT [768x4096 fp8] ----
            with tc.tile_pool(name="psB", bufs=3, space="PSUM") as psb:
                pb = pa
                wq_sb = pb.tile([P, DT, D], f8, tag="wq_sb")
                nc.sync.dma_start(wq_sb[:], wq8[:])
                nc.sync.dma_start(bq_sb[:], bq[:])
                nc.sync.dma_start(wc_sb[:], wcT[:])
                nc.sync.dma_start(bc_sb[:], bc[:])
                nc.gpsimd.partition_broadcast(bcb[:], bc_sb[:])
                for c in range(NQC):
                    x8c = pb.tile([P, DT, QC], f8, tag="xq8c", bufs=3, name="xq8c")
                    nc.sync.dma_start(x8c[:], xq8[:, c * CHB:(c + 1) * CHB])
                    for go in range(DT):
                        ps = psb.tile([P, QC], f32, tag="pq")
                        for a in range(3):
                            nc.tensor.matmul(
                                ps[:], wq_sb[:, 2 * a:2 * a + 2, go * P:(go + 1) * P],
                                x8c[:, 2 * a:2 * a + 2, :],
                                start=(a == 0), stop=(a == 2), perf_mode=DR)
                        nc.vector.tensor_scalar_add(
                            qT_sb[:, c, go, :], ps[:],
                            bq_sb[:, go:go + 1])
                _ab_stack.close()

            # ---- Phase C: attention; yTaug; chunked RS; late norm ----
            with tc.tile_pool(name="pC", bufs=2) as pc, \
                 tc.tile_pool(name="pE", bufs=2) as pe:
                f_view = f_dram[:].rearrange("a b -> (a b)").rearrange(
                    "(x c) -> x c", c=S)

                def norm_head(b, q):
                    s_row = pe.tile([1, QC], bf16, tag="s_row", name="s_row")
                    q.dma_start(s_row[:], rs_out[b][FH:FH + 1, :])
                    rsb = pe.tile([P, 3, QC], bf16, tag="rsb", bufs=2, name="rsb")
                    q.dma_start(
                        rsb[:], rs_out[b][0:FH, :].rearrange("(r p) c -> p r c", p=P))
                    den = pe.tile([P, QC], bf16, tag="den", name="den")
                    nc.gpsimd.partition_broadcast(den[:], s_row[:])
                    den32 = pe.tile([P, QC], f32, tag="den32", name="den32")
                    nc.vector.tensor_copy(den32[:], den[:])
                    rec = pe.tile([P, QC], f32, tag="rec", name="rec", bufs=2)
                    nc.vector.reciprocal_approx_fast(rec[:], den32[:])
                    return rsb, rec

                def norm_r(b, rsb, rec, r, q):
                    tmp = pe.tile([P, QC], f32, tag="tmp", bufs=2, name="tmp")
                    nc.vector.tensor_mul(tmp[:], rsb[:, r, :], rec[:])
                    fn = pe.tile([P, QC], bf16, tag="fn", bufs=2, name="fn")
                    nc.vector.tensor_scalar_add(fn[:], tmp[:], bvh_sb[:, r:r + 1])
                    q.dma_start(
                        f_view[r * P:(r + 1) * P, b * QC:(b + 1) * QC], fn[:])

                def emit_norm(b, q):
                    rsb, rec = norm_head(b, q)
                    for r in range(FH // P):
                        norm_r(b, rsb, rec, r, q)

                pending = []
                psc_ctx = tc.tile_pool(name="psC", bufs=1, space="PSUM")
                psc = psc_ctx.__enter__()
                for qc in range(NQC):
                    sums_acc = pc.tile([P, QC], bf16, tag="sums_acc")
                    nc.vector.memset(sums_acc[:], 0.0)
                    ypsum = [psc.tile([P, QC], f32, name=f"y{e}", tag=f"y{e}", bufs=1)
                             for e in range(DT)]
                    a_pairs = {}
                    for kt in range(KT):
                        if kt == 3 and pending:
                            emit_norm(pending.pop(0), nc.sync)
                        aps = psc.tile([P, QC], f32, tag="att", bufs=2)
                        for a in range(3):
                            nc.tensor.matmul(
                                aps[:], kT_sb[:, 2 * a:2 * a + 2, kt * P:(kt + 1) * P],
                                qT_sb[:, qc, 2 * a:2 * a + 2, :],
                                start=(a == 0), stop=(a == 2), perf_mode=DR)
                        if kt % 2 == 0:
                            a_pairs[kt // 2] = pc.tile([P, 2, QC], f8, tag="a_pr",
                                                       bufs=4, name="a_pr")
                            # av matmuls for the previous key-tile pair issue
                            # while this kt's exp is still on the scalar engine
                            if kt > 0:
                                pr = kt // 2 - 1
                                for e in range(DT):
                                    nc.tensor.matmul(
                                        ypsum[e][:],
                                        v_sb[:, 2 * pr:2 * pr + 2, e * P:(e + 1) * P],
                                        a_pairs[pr][:],
                                        start=(pr == 0), stop=False, perf_mode=DR)
                        a_half = a_pairs[kt // 2][:, kt % 2, :]
                        nc.scalar.activation(a_half, aps[:], Exp, scale=EXP_SCALE)
                        nc.vector.tensor_add(sums_acc[:], sums_acc[:], a_half)
                    pr = NPR - 1
                    for e in range(DT):
                        nc.tensor.matmul(
                            ypsum[e][:], v_sb[:, 2 * pr:2 * pr + 2, e * P:(e + 1) * P],
                            a_pairs[pr][:],
                            start=False, stop=True, perf_mode=DR)
                    # ones.T @ sums_acc reduces across partitions and
                    # replicates the result onto all 128 partitions
                    sp = psc.tile([P, QC], f32, tag="att", bufs=2)
                    nc.tensor.matmul(sp[:], ones_sb[:], sums_acc[:], start=True, stop=True)
                    yb = yTaug[qc]
                    # drain ypsum banks in consumption order, alternating
                    # Scalar/Vector so the next qc's av matmuls free up fast
                    for e in range(DT):
                        yt_sb = pc.tile([P, QC], bf16, tag="yt_sb", bufs=4)
                        if e % 2 == 0:
                            nc.scalar.activation(yt_sb[:], ypsum[e][:], Copy)
                        else:
                            nc.vector.tensor_copy(yt_sb[:], ypsum[e][:])
                        row = e * P if e < 3 else (FH + 1) + (e - 3) * P
                        nc.sync.dma_start(yb[row:row + P, :], yt_sb[:])
                    sbc = pc.tile([P, QC], bf16, tag="sbc")
                    nc.vector.tensor_copy(sbc[:], sp[:])
                    nc.sync.dma_start(yb[FH:FH + 1, :], sbc[0:1, :])
                    nc.sync.dma_start(yb[2 * FH + 1:2 * FH + 2, :], sbc[0:1, :])

                    nc.gpsimd.collective_compute(
                        "ReduceScatter", mybir.AluOpType.add,
                        replica_groups=GROUPS,
                        ins=[yTaug[qc].opt()], outs=[rs_out[qc].opt()])
                    if qc > 0:
                        pending.append(qc - 1)

                psc_ctx.__exit__(None, None, None)
                for b in pending:
                    emit_norm(b, nc.scalar)
                emit_norm(NQC - 1, nc.scalar)

                # ---- Phase F: out = y_perm @ Wc.T + bc ----
                with tc.tile_pool(name="pF", bufs=1) as pf, \
                     tc.tile_pool(name="psF", bufs=2, space="PSUM") as psf:
                    for tb in range(SK // QC):
                        fT = pf.tile([P, DT, QC], bf16, tag="fT", bufs=4, name="fT")
                        nc.sync.dma_start_transpose(
                            fT[:], f_dram[tb * QC:(tb + 1) * QC, :])
                        for u in range(4):
                            t = tb * 4 + u
                            po = psf.tile([P, QC], f32, tag="po")
                            po2 = psf.tile([P, D - QC], f32, tag="po2")
                            for gi in range(DT):
                                nc.tensor.matmul(po[:], fT[:, gi, u * P:(u + 1) * P],
                                                 wc_sb[:, gi, 0:QC],
                                                 start=(gi == 0), stop=(gi == DT - 1))
                                nc.tensor.matmul(po2[:], fT[:, gi, u * P:(u + 1) * P],
                                                 wc_sb[:, gi, QC:D],
                                                 start=(gi == 0), stop=(gi == DT - 1))
                            o_sb = pf.tile([P, D], f32, tag="o_sb", bufs=3)
                            nc.vector.tensor_add(o_sb[:, 0:QC], po[:], bcb[:, 0:QC])
                            nc.vector.tensor_add(o_sb[:, QC:D], po2[:], bcb[:, QC:D])
                            # out-writes go on the scalar HWDGE queue so the
                            # sync queue stays free for the fT transposes
                            nc.scalar.dma_start(out[t * P:(t + 1) * P, :], o_sb[:])

    return nc


def _get_nc():
    global _nc
    if _nc is None:
        _nc = _build_program()
        _nc.finalize()
    return _nc


def _to_f8(a):
    return np.clip(a, -240.0, 240.0).astype(F8)


def _shuffle_w(w):
    # [768, 768] -> [128, 6*768]: row g*128+p lands at [p, g, :]
    return np.ascontiguousarray(
        w.reshape(DT, P, D).transpose(1, 0, 2)).reshape(P, DT * D)


def _shuffle_x(xT, nch):
    # [768, nch*512] -> [128, nch*6*512]: chunk-major, feature-tile, column
    return np.ascontiguousarray(
        xT.reshape(DT, P, nch, QC).transpose(1, 2, 0, 3)).reshape(P, nch * DT * QC)


def _prep_in_maps(x, Wq, bq, Wk, bk, Wv, bv, Wc, bc):
    x = np.asarray(x, dtype=np.float32)
    wq8 = _shuffle_w(_to_f8(np.asarray(Wq, np.float32).T * WS))
    wk8 = _shuffle_w(_to_f8(np.asarray(Wk, np.float32).T * WS))
    wvT = _shuffle_w(np.asarray(Wv, np.float32).T.astype(BF16))
    wcT = _shuffle_w(np.asarray(Wc, np.float32).T.astype(BF16))
    bqc = np.ascontiguousarray((np.asarray(bq, np.float32) * WS).reshape(DT, P).T)
    bkc = np.ascontiguousarray((np.asarray(bk, np.float32) * WS).reshape(DT, P).T)
    bvc = np.asarray(bv, np.float32).reshape(D)
    bcc = np.asarray(bc, np.float32).reshape(1, D).copy()
    in_maps = []
    for c in range(8):
        b, h = divmod(c, 2)
        xT = np.ascontiguousarray(x[b].T)          # [D, S]
        xT8 = _to_f8(xT)
        kvT8 = xT8[:, h * SK:(h + 1) * SK]
        kvT16 = xT[:, h * SK:(h + 1) * SK].astype(BF16)
        in_maps.append({
            "xq8": _shuffle_x(xT8, NQC),
            "xkv8": _shuffle_x(kvT8, NKC),
            "xkv16": _shuffle_x(kvT16, NKC),
            "wq8": wq8, "wk8": wk8, "wvT": wvT, "wcT": wcT,
            "bq": bqc, "bk": bkc,
            "bvh": np.ascontiguousarray(bvc[h * FH:(h + 1) * FH].reshape(3, P).T),
            "bc": bcc,
        })
    return in_maps


def _assemble(results):
    out = np.empty((B, S, D), dtype=np.float32)
    for c in range(8):
        b, h = divmod(c, 2)
        out[b, h * SK:(h + 1) * SK, :] = results[c]["out"]
    return out


def run_on_hw(trace=False, **inputs):
    from concourse.bass_utils import run_bass_kernel_spmd
    nc = _get_nc()
    in_maps = _prep_in_maps(**inputs)
    res = run_bass_kernel_spmd(nc, in_maps, list(range(8)), trace=trace)
    return _assemble(res.results), res


def kernel(**inputs):
    out, _ = run_on_hw(trace=False, **inputs)
    return out


# revision 21
# speedup vs baseline: 1.2003x; 1.0461x over previous
"""Trainium2 Bass kernel for single-head full-dim attention (nn_CasualSelfAttention).

Reference math (B=4, S=4096, D=768, fp32):
    q = x @ Wq.T + bq ; k = x @ Wk.T + bk ; v = x @ Wv.T + bv
    att = softmax(q @ k.T * D**-0.5)        # no mask
    y = att @ v
    y = y.transpose(0,2,1).reshape(B,S,D)   # element permutation
    out = y @ Wc.T + bc

Sharding (8 cores): core c = 2*b + h handles batch b with ALL 4096 queries but
only its half of the keys/values (rows h*2048:(h+1)*2048). Each core produces a
partial unnormalized yT [768, 4096] (features x queries) plus partial softmax
sums. A pairwise ReduceScatter(add), chunked by 512-query column blocks, hands
core h the fully-reduced feature slice [384*h : 384*h+384] for all queries —
exactly the rows of y.T that the permutation maps to output rows
[2048*h : 2048*h+2048]. After normalizing by the (also-reduced) sums and adding
bv (valid because sum(att)=den), the flat buffer IS y_perm row-major, and the
final projection runs locally.

Performance structure (v5):
  - All inputs host-pre-shuffled to [128-partition, ...] contiguous layouts so
    every device load is a max-rate plain DMA.
  - q/k projections, q.k^T, AND att@v run as fp8e4m3 DoubleRow matmuls
    (2 contraction k-tiles of 128 per matmul). The DR moving operand keeps its
    k-pair at a small stride (<=768B) — large j-strides halve the issue rate.
  - v/c projections stay bf16 (fp8 there pushes output error past the gate);
    v is computed in bf16 then quantized to fp8 for the av matmul.
  - ReduceScatter payloads are bf16; softmax sums ride in yTaug rows 384/769;
    a full-size dummy collective at kernel start absorbs first-RS warmup.
  - bv is folded in post-normalization (num/den + bv).
  - Norm for block b is emitted a few instructions into qc b+2, with its DMAs
    on the sync queue (scalar-queue loads would block the exp stream).
  - Tail: the last block's norm runs on the scalar queue right after the last
    RS, then phase F streams 4 transposed loads (bufs=4) against its matmuls.
"""

import numpy as np
import ml_dtypes

BF16 = ml_dtypes.bfloat16
F8 = ml_dtypes.float8_e4m3

B, S, D = 4, 4096, 768
SK = S // 2            # keys per core
P = 128
DT = D // P            # 6 feature tiles
KT = SK // P           # 16 key tiles
NPR = KT // 2          # 8 key-tile pairs for the av DoubleRow matmuls
QC = 512               # query chunk width
NQC = S // QC          # 8 query chunks / RS blocks
NKC = SK // QC         # 4 key chunks
FH = D // 2            # 384: feature rows per RS chunk
WS = 32.0              # host-side fp8 weight scale for Wq/Wk (and bq/bk)
SCALE = float(D) ** -0.5
EXP_SCALE = SCALE / (WS * WS)
GROUPS = [[0, 1], [2, 3], [4, 5], [6, 7]]

_nc = None


def _build_program():
    import concourse.bass as bass
    import concourse.mybir as mybir
    import concourse.tile as tile
    from concourse import bacc

    f32 = mybir.dt.float32
    bf16 = mybir.dt.bfloat16
    f8 = mybir.dt.float8e4
    Exp = mybir.ActivationFunctionType.Exp
    Copy = mybir.ActivationFunctionType.Copy
    DR = mybir.MatmulPerfMode.DoubleRow

    nc = bacc.Bacc(None, num_devices=8)

    # all x/weight layouts are host-pre-shuffled to partition-major flat 2D
    xq8 = nc.declare_dram_parameter("xq8", [P, NQC * DT * QC], f8, isOutput=False)
    xkv8 = nc.declare_dram_parameter("xkv8", [P, NKC * DT * QC], f8, isOutput=False)
    xkv16 = nc.declare_dram_parameter("xkv16", [P, NKC * DT * QC], bf16, isOutput=False)
    wq8 = nc.declare_dram_parameter("wq8", [P, DT * D], f8, isOutput=False)
    wk8 = nc.declare_dram_parameter("wk8", [P, DT * D], f8, isOutput=False)
    wvT = nc.declare_dram_parameter("wvT", [P, DT * D], bf16, isOutput=False)
    wcT = nc.declare_dram_parameter("wcT", [P, DT * D], bf16, isOutput=False)
    bq = nc.declare_dram_parameter("bq", [P, DT], f32, isOutput=False)
    bk = nc.declare_dram_parameter("bk", [P, DT], f32, isOutput=False)
    bvh = nc.declare_dram_parameter("bvh", [P, 3], f32, isOutput=False)
    bc = nc.declare_dram_parameter("bc", [1, D], f32, isOutput=False)
    out = nc.declare_dram_parameter("out", [SK, D], f32, isOutput=True)

    CHB = DT * QC  # 3072: one 512-col chunk of x per partition row

    with tile.TileContext(nc) as tc:
        with tc.tile_pool(name="persist", bufs=1) as pp, \
             tc.tile_pool(name="dram", bufs=1, space="DRAM") as dram:
            # Per column block: rows 0:384 = feats 0:384, row 384 = partial
            # sums, rows 385:769 = feats 384:768, row 769 = partial sums.
            yTaug = [dram.tile([2 * (FH + 1), QC], bf16, name=f"yTaug{b}", tag=f"yTaug{b}")
                     for b in range(NQC)]
            rs_out = [dram.tile([FH + 1, QC], bf16, name=f"rs_out{b}", tag=f"rs_out{b}")
                      for b in range(NQC)]
            f_dram = dram.tile([SK, D], bf16)
            # realistically-sized dummy collective to absorb first-RS warmup
            # (contents uninitialized; output unused)
            warm_in = dram.tile([2 * (FH + 1), QC], bf16, name="warm_in", tag="warm_in")
            warm_out = dram.tile([FH + 1, QC], bf16, name="warm_out", tag="warm_out")

            # persistent SBUF. qT keeps each 512-query chunk's six feature
            # tiles contiguous; v keeps the key-tile pairs at stride 768B.
            kT_sb = pp.tile([P, DT, SK], f8, tag="kT")
            qT_sb = pp.tile([P, NQC, DT, QC], f8, tag="qT")
            v_sb = pp.tile([P, KT, D], f8, tag="v_sb")
            bq_sb = pp.tile([P, DT], f32, tag="bq_sb")
            bk_sb = pp.tile([P, DT], f32, tag="bk_sb")
            bvh_sb = pp.tile([P, 3], f32, tag="bvh_sb")
            ones_sb = pp.tile([P, P], bf16, name="ones", tag="ones")
            wc_sb = pp.tile([P, DT, D], bf16, tag="wc_sb")
            bc_sb = pp.tile([1, D], f32, tag="bc_sb")
            bcb = pp.tile([P, D], f32, tag="bcb")

            # ---- Phase A: kT [768x2048 fp8] and v [2048x768 fp8] ----
            import contextlib
            _ab_stack = contextlib.ExitStack()
            pa = _ab_stack.enter_context(tc.tile_pool(name="pA", bufs=1))
            with tc.tile_pool(name="psA", bufs=2, space="PSUM") as psa:
                wk_sb = pa.tile([P, DT, D], f8, tag="wk_sb")
                nc.sync.dma_start(wk_sb[:], wk8[:])
                x8cs = []
                for c in range(NKC):
                    x8c = pa.tile([P, DT, QC], f8, tag=f"x8c{c}", name=f"x8c{c}")
                    nc.sync.dma_start(x8c[:], xkv8[:, c * CHB:(c + 1) * CHB])
                    x8cs.append(x8c)
                    if c == 0:
                        nc.sync.dma_start(bk_sb[:], bk[:])
                wv_sb = pa.tile([P, DT, D], bf16, tag="wv_sb")
                nc.sync.dma_start(wv_sb[:], wvT[:])
                x16c0 = pa.tile([P, DT, QC], bf16, tag="x16c0", name="x16c0")
                nc.sync.dma_start(x16c0[:], xkv16[:, 0:CHB])
                # all k-proj chunks back-to-back (pure fp8-DR, no mode mixing)
                for c in range(NKC):
                    for go in range(DT):
                        ps = psa.tile([P, QC], f32, tag="pk")
                        for a in range(3):
                            nc.tensor.matmul(
                                ps[:], wk_sb[:, 2 * a:2 * a + 2, go * P:(go + 1) * P],
                                x8cs[c][:, 2 * a:2 * a + 2, :],
                                start=(a == 0), stop=(a == 2), perf_mode=DR)
                        nc.vector.tensor_scalar_add(
                            kT_sb[:, go, c * QC:(c + 1) * QC], ps[:],
                            bk_sb[:, go:go + 1])
                # warm up the collectives stack while A computes
                nc.gpsimd.collective_compute(
                    "ReduceScatter", mybir.AluOpType.add,
                    replica_groups=GROUPS,
                    ins=[warm_in[:].opt()], outs=[warm_out[:].opt()])
                nc.vector.memset(ones_sb[:], 1.0)
                nc.sync.dma_start(bvh_sb[:], bvh[:])
                for c in range(NKC):
                    if c == 0:
                        x16c = x16c0
                    else:
                        x16c = pa.tile([P, DT, QC], bf16, tag="x16c", bufs=2, name="x16c")
                        nc.sync.dma_start(x16c[:], xkv16[:, c * CHB:(c + 1) * CHB])
                    for tl in range(4):
                        t = c * 4 + tl
                        for half in range(2):
                            ps = psa.tile([P, FH], f32, tag="pv")
                            for gi in range(DT):
                                nc.tensor.matmul(
                                    ps[:], x16c[:, gi, tl * P:(tl + 1) * P],
                                    wv_sb[:, gi, half * FH:(half + 1) * FH],
                                    start=(gi == 0), stop=(gi == DT - 1))
                            nc.vector.tensor_copy(
                                v_sb[:, t, half * FH:(half + 1) * FH], ps[:])

            # ---- Phase B: qT [768x4096 fp8] ----
            with tc.tile_pool(name="psB", bufs=3, space="PSUM") as psb:
                pb = pa
                wq_sb = pb.tile([P, DT, D], f8, tag="wq_sb")
                nc.sync.dma_start(wq_sb[:], wq8[:])
                nc.sync.dma_start(bq_sb[:], bq[:])
                nc.sync.dma_start(wc_sb[:], wcT[:])
                nc.sync.dma_start(bc_sb[:], bc[:])
                nc.gpsimd.partition_broadcast(bcb[:], bc_sb[:])
                for c in range(NQC):
                    x8c = pb.tile([P, DT, QC], f8, tag="xq8c", bufs=3, name="xq8c")
                    nc.sync.dma_start(x8c[:], xq8[:, c * CHB:(c + 1) * CHB])
                    for go in range(DT):
                        ps = psb.tile([P, QC], f32, tag="pq")
                        for a in range(3):
                            nc.tensor.matmul(
                                ps[:], wq_sb[:, 2 * a:2 * a + 2, go * P:(go + 1) * P],
                                x8c[:, 2 * a:2 * a + 2, :],
                                start=(a == 0), stop=(a == 2), perf_mode=DR)
                        nc.vector.tensor_scalar_add(
                            qT_sb[:, c, go, :], ps[:],
                            bq_sb[:, go:go + 1])
                _ab_stack.close()

            # ---- Phase C: attention; yTaug; chunked RS; late norm ----
            with tc.tile_pool(name="pC", bufs=2) as pc, \
                 tc.tile_pool(name="pE", bufs=2) as pe:
                f_view = f_dram[:].rearrange("a b -> (a b)").rearrange(
                    "(x c) -> x c", c=S)

                def norm_head(b, q):
                    # casting SWDGE load: bf16 sums row -> f32 (gpsimd is idle,
                    # and reciprocal_approx_fast needs fp32 input)
                    s_row = pe.tile([1, QC], f32, tag="s_row", name="s_row")
                    nc.gpsimd.dma_start(s_row[:], rs_out[b][FH:FH + 1, :])
                    rsb = pe.tile([P, 3, QC], bf16, tag="rsb", bufs=2, name="rsb")
                    q.dma_start(
                        rsb[:], rs_out[b][0:FH, :].rearrange("(r p) c -> p r c", p=P))
                    den32 = pe.tile([P, QC], f32, tag="den32", name="den32")
                    nc.gpsimd.partition_broadcast(den32[:], s_row[:])
                    rec = pe.tile([P, QC], f32, tag="rec", name="rec", bufs=2)
                    nc.vector.reciprocal_approx_fast(rec[:], den32[:])
                    return rsb, rec

                def norm_r(b, rsb, rec, r, q):
                    tmp = pe.tile([P, QC], f32, tag="tmp", bufs=2, name="tmp")
                    nc.vector.tensor_mul(tmp[:], rsb[:, r, :], rec[:])
                    fn = pe.tile([P, QC], bf16, tag="fn", bufs=2, name="fn")
                    nc.vector.tensor_scalar_add(fn[:], tmp[:], bvh_sb[:, r:r + 1])
                    q.dma_start(
                        f_view[r * P:(r + 1) * P, b * QC:(b + 1) * QC], fn[:])

                def emit_norm(b, q):
                    rsb, rec = norm_head(b, q)
                    for r in range(FH // P):
                        norm_r(b, rsb, rec, r, q)

                pending = []
                psc_ctx = tc.tile_pool(name="psC", bufs=1, space="PSUM")
                psc = psc_ctx.__enter__()

                def drain_copy(ypsum, yb, e, on_act):
                    yt_sb = pc.tile([P, QC], bf16, tag="yt_sb", bufs=4)
                    if on_act:
                        nc.scalar.activation(yt_sb[:], ypsum[e][:], Copy)
                    else:
                        nc.vector.tensor_copy(yt_sb[:], ypsum[e][:])
                    row = e * P if e < 3 else (FH + 1) + (e - 3) * P
                    nc.sync.dma_start(yb[row:row + P, :], yt_sb[:])

                def finish_block(pv):
                    ypsum, sbc, yb, b = pv
                    nc.sync.dma_start(yb[FH:FH + 1, :], sbc[0:1, :])
                    nc.sync.dma_start(yb[2 * FH + 1:2 * FH + 2, :], sbc[0:1, :])
                    nc.gpsimd.collective_compute(
                        "ReduceScatter", mybir.AluOpType.add,
                        replica_groups=GROUPS,
                        ins=[yTaug[b].opt()], outs=[rs_out[b].opt()])
                    if b > 0:
                        pending.append(b - 1)

                prev = None
                for qc in range(NQC):
                    sums_acc = pc.tile([P, QC], bf16, tag="sums_acc")
                    nc.vector.memset(sums_acc[:], 0.0)
                    ypsum = [psc.tile([P, QC], f32, name=f"y{e}", tag=f"y{e}", bufs=1)
                             for e in range(DT)]
                    a_pairs = {}
                    for kt in range(KT):
                        # the previous qc's PSUM drain is spread over this qc's
                        # first kt slots so it never delays this qc's exps
                        if prev is not None and kt < 2:
                            for j in range(3):
                                e = 3 * kt + j
                                drain_copy(prev[0], prev[2], e, on_act=(e % 2 == 0))
                        if kt == 2 and prev is not None:
                            finish_block(prev)
                            prev = None
                        if kt == 3 and pending:
                            emit_norm(pending.pop(0), nc.sync)
                        aps = psc.tile([P, QC], f32, tag="att", bufs=2)
                        for a in range(3):
                            nc.tensor.matmul(
                                aps[:], kT_sb[:, 2 * a:2 * a + 2, kt * P:(kt + 1) * P],
                                qT_sb[:, qc, 2 * a:2 * a + 2, :],
                                start=(a == 0), stop=(a == 2), perf_mode=DR)
                        if kt % 2 == 0:
                            a_pairs[kt // 2] = pc.tile([P, 2, QC], f8, tag="a_pr",
                                                       bufs=4, name="a_pr")
                            # av matmuls for the previous key-tile pair issue
                            # while this kt's exp is still on the scalar engine
                            if kt > 0:
                                pr = kt // 2 - 1
                                for e in range(DT):
                                    nc.tensor.matmul(
                                        ypsum[e][:],
                                        v_sb[:, 2 * pr:2 * pr + 2, e * P:(e + 1) * P],
                                        a_pairs[pr][:],
                                        start=(pr == 0), stop=False, perf_mode=DR)
                        a_half = a_pairs[kt // 2][:, kt % 2, :]
                        nc.scalar.activation(a_half, aps[:], Exp, scale=EXP_SCALE)
                        nc.vector.tensor_add(sums_acc[:], sums_acc[:], a_half)
                    pr = NPR - 1
                    for e in range(DT):
                        nc.tensor.matmul(
                            ypsum[e][:], v_sb[:, 2 * pr:2 * pr + 2, e * P:(e + 1) * P],
                            a_pairs[pr][:],
                            start=False, stop=True, perf_mode=DR)
                    # ones.T @ sums_acc reduces across partitions and
                    # replicates the result onto all 128 partitions
                    sp = psc.tile([P, QC], f32, tag="att", bufs=2)
                    nc.tensor.matmul(sp[:], ones_sb[:], sums_acc[:], start=True, stop=True)
                    # copy the sums off PSUM now (the "att" bank recycles at
                    # kt1 of the next qc); the DMAs + RS trigger are deferred
                    sbc = pc.tile([P, QC], bf16, tag="sbc", bufs=2)
                    nc.vector.tensor_copy(sbc[:], sp[:])
                    prev = (ypsum, sbc, yTaug[qc], qc)

                # last qc: drain immediately (nothing left to overlap with)
                for e in range(DT):
                    drain_copy(prev[0], prev[2], e, on_act=(e % 2 == 0))
                finish_block(prev)
                psc_ctx.__exit__(None, None, None)
                for b in pending:
                    emit_norm(b, nc.scalar)
                emit_norm(NQC - 1, nc.scalar)

                # ---- Phase F: out = y_perm @ Wc.T + bc ----
                # All four transposed loads are issued before any out-write:
                # Tile serializes DMA-transposes against other in-flight DMA
                # traffic (HW deadlock guard), so interleaving them with the
                # out-writes stalls both queues.
                with tc.tile_pool(name="pF", bufs=1) as pf, \
                     tc.tile_pool(name="psF", bufs=2, space="PSUM") as psf:
                    fTs = []
                    for tb in range(SK // QC):
                        fT = pf.tile([P, DT, QC], bf16, tag=f"fT{tb}", name=f"fT{tb}")
                        nc.sync.dma_start_transpose(
                            fT[:], f_dram[tb * QC:(tb + 1) * QC, :])
                        fTs.append(fT)
                    for tb in range(SK // QC):
                        fT = fTs[tb]
                        for u in range(4):
                            t = tb * 4 + u
                            po = psf.tile([P, QC], f32, tag="po")
                            po2 = psf.tile([P, D - QC], f32, tag="po2")
                            for gi in range(DT):
                                nc.tensor.matmul(po[:], fT[:, gi, u * P:(u + 1) * P],
                                                 wc_sb[:, gi, 0:QC],
                                                 start=(gi == 0), stop=(gi == DT - 1))
                                nc.tensor.matmul(po2[:], fT[:, gi, u * P:(u + 1) * P],
                                                 wc_sb[:, gi, QC:D],
                                                 start=(gi == 0), stop=(gi == DT - 1))
                            o_sb = pf.tile([P, D], f32, tag="o_sb", bufs=8)
                            nc.vector.tensor_add(o_sb[:, 0:QC], po[:], bcb[:, 0:QC])
                            nc.vector.tensor_add(o_sb[:, QC:D], po2[:], bcb[:, QC:D])
                            # out-writes go on the scalar HWDGE queue so the
                            # sync queue stays free for the fT transposes
                            nc.scalar.dma_start(out[t * P:(t + 1) * P, :], o_sb[:])

    return nc


def _get_nc():
    global _nc
    if _nc is None:
        _nc = _build_program()
        _nc.finalize()
    return _nc


def _to_f8(a):
    return np.clip(a, -240.0, 240.0).astype(F8)


def _shuffle_w(w):
    # [768, 768] -> [128, 6*768]: row g*128+p lands at [p, g, :]
    return np.ascontiguousarray(
        w.reshape(DT, P, D).transpose(1, 0, 2)).reshape(P, DT * D)


def _shuffle_x(xT, nch):
    # [768, nch*512] -> [128, nch*6*512]: chunk-major, feature-tile, column
    return np.ascontiguousarray(
        xT.reshape(DT, P, nch, QC).transpose(1, 2, 0, 3)).reshape(P, nch * DT * QC)


def _prep_in_maps(x, Wq, bq, Wk, bk, Wv, bv, Wc, bc):
    x = np.asarray(x, dtype=np.float32)
    wq8 = _shuffle_w(_to_f8(np.asarray(Wq, np.float32).T * WS))
    wk8 = _shuffle_w(_to_f8(np.asarray(Wk, np.float32).T * WS))
    wvT = _shuffle_w(np.asarray(Wv, np.float32).T.astype(BF16))
    wcT = _shuffle_w(np.asarray(Wc, np.float32).T.astype(BF16))
    bqc = np.ascontiguousarray((np.asarray(bq, np.float32) * WS).reshape(DT, P).T)
    bkc = np.ascontiguousarray((np.asarray(bk, np.float32) * WS).reshape(DT, P).T)
    bvc = np.asarray(bv, np.float32).reshape(D)
    bcc = np.asarray(bc, np.float32).reshape(1, D).copy()
    in_maps = []
    for c in range(8):
        b, h = divmod(c, 2)
        xT = np.ascontiguousarray(x[b].T)          # [D, S]
        xT8 = _to_f8(xT)
        kvT8 = xT8[:, h * SK:(h + 1) * SK]
        kvT16 = xT[:, h * SK:(h + 1) * SK].astype(BF16)
        in_maps.append({
            "xq8": _shuffle_x(xT8, NQC),
            "xkv8": _shuffle_x(kvT8, NKC),
            "xkv16": _shuffle_x(kvT16, NKC),
            "wq8": wq8, "wk8": wk8, "wvT": wvT, "wcT": wcT,
            "bq": bqc, "bk": bkc,
            "bvh": np.ascontiguousarray(bvc[h * FH:(h + 1) * FH].reshape(3, P).T),
            "bc": bcc,
        })
    return in_maps


def _assemble(results):
    out = np.empty((B, S, D), dtype=np.float32)
    for c in range(8):
        b, h = divmod(c, 2)
        out[b, h * SK:(h + 1) * SK, :] = results[c]["out"]
    return out


def run_on_hw(trace=False, **inputs):
    from concourse.bass_utils import run_bass_kernel_spmd
    nc = _get_nc()
    in_maps = _prep_in_maps(**inputs)
    res = run_bass_kernel_spmd(nc, in_maps, list(range(8)), trace=trace)
    return _assemble(res.results), res


def kernel(**inputs):
    out, _ = run_on_hw(trace=False, **inputs)
    return out
